# revision 92
# baseline (speedup 1.0000x reference)
"""Bass/Tile Trainium2 kernel for the additive-attention module.

reference (per batch row b):
    q = hidden_state @ Wa.T + ba                 # [A]
    k = feature_vectors[b] @ Ua.T                # [L, A]
    e = tanh(q + k) @ w                          # [L]
    attn = softmax(e)                            # [L]
    context[b] = attn @ feature_vectors[b]       # [M]

Sharding: data-parallel over batch B=64 -> 8 cores x 8 rows, params
replicated, no collectives. Each core streams its 32 MB feature_vector
shard from HBM exactly once.

Precision: fp32 matmuls cost 4 cycles/column on TRN2 PE (two half-rate
passes); fp16 costs 1. The fv pipeline (fv, Ua, tanh output, attn
weights) runs in fp16 (10 mantissa bits; measured end-to-end rel err
2.7e-3 vs 1.8e-2 for bf16); softmax statistics and all accumulations
(PSUM) stay fp32.

Per-core dataflow (per batch row):
  - fv cast fp32->fp16 during the HBM DMA (SWDGE), natural [l, m] layout
  - PE transposes 128x128 fp16 tiles of fv into [m, l] layout (PSUM),
    DVE evacuates; k-matmul streams fv.T columns with Ua.T stationary
  - ScalarE evacuates the k PSUM with fused per-partition bias q[a] and
    tanh in one ACTIVATE (fp16 out)
  - e = w.T @ tanh(...) on PE; softmax: DRAM-bounce reshape of e
    [1,4096] -> [128,32], DVE row max, GPSIMD cross-partition max, ACT
    exp with accum_out row sums, GPSIMD cross-partition sum
  - weighted sum on PE: attn column [128,1] fp16 stationary, natural
    fv tiles streaming; denominator applied at the end in fp32
  - softmax+weighted-sum of row b are emitted only after row b+2's main
    stage: the bounce readback can run a full row late under fv-load DMA
    congestion, and with less slack a late readback head-of-line-blocks
    the in-order DVE queue (the reduce_max waits on it, row b+1's fvT
    evacuations queue behind, and the PE k-matmuls starve)

Scheduling (the difference between 220us and 190us):
  - param DMAs are emitted before any fv load so they reach the DMA
    engines before the SWDGE stream saturates them; hs is loaded
    natural+PE-transposed (a 4-byte-gather AP starves under load)
  - identities are built before the fv descgens on the serial gpsimd
    queue (they gate every transpose)
  - row b+1's fv load is emitted at the TOP of main_stage(b), ahead of
    row b's all-reduces on the same gpsimd queue; otherwise each load
    starts only after AR(b) fires mid-row b+1 and the transposes chase
    the DMA with zero margin
  - the e bounce is 2 half-writes + 2 half-readbacks pipelined inside
    the row (the sync SEQ spends ~1.2us per DMA; 10 issues/row
    saturates it), quarters on the last row to shrink the tail wait
  - the last row's softmax reductions run on a PE-transpose/DVE chain
    instead of gpsimd all-reduces (lower latency, PE is idle there)
  - deeper prefetch or faster (32KB-descriptor) loads measure WORSE:
    sustained-DMA bursts trip the chip power cap and throttle all
    clocks, and bulk DMA starves the small latency-critical transfers
"""

import numpy as np

B, R, M, A, L = 64, 512, 256, 256, 4096
NCORES = 8
BLOC = B // NCORES  # 8 batch rows per core
NL = L // 128  # 32 l-chunks of 128
NJG = 8  # j-groups of 512 l-columns
JW = L // NJG  # 512

_CACHE = {}


def _build():
    from contextlib import ExitStack

    import concourse.bacc as bacc
    import concourse.bass as bass
    import concourse.bass_isa as bass_isa
    import concourse.mybir as mybir
    import concourse.tile as tile
    from concourse.masks import make_identity

    f32 = mybir.dt.float32
    f16 = mybir.dt.float16
    AF = mybir.ActivationFunctionType

    nc = bacc.Bacc("TRN2", target_bir_lowering=False, debug=False,
                   num_devices=NCORES)

    hs = nc.dram_tensor("hidden_state", [BLOC, R], f32, kind="ExternalInput").ap()
    fv = nc.dram_tensor("feature_vectors", [BLOC, L, M], f32,
                        kind="ExternalInput").ap()
    Wa = nc.dram_tensor("Wa", [A, R], f32, kind="ExternalInput").ap()
    Ua = nc.dram_tensor("Ua", [A, M], f32, kind="ExternalInput").ap()
    w = nc.dram_tensor("w", [A, 1], f32, kind="ExternalInput").ap()
    ba = nc.dram_tensor("ba", [1, A], f32, kind="ExternalInput").ap()
    ctx_out = nc.dram_tensor("context", [BLOC, M], f32, kind="ExternalOutput").ap()

    with tile.TileContext(nc) as tc, ExitStack() as ctx:
        singles = ctx.enter_context(tc.tile_pool(name="singles", bufs=1))
        ldpool = ctx.enter_context(tc.tile_pool(name="ldpool", bufs=2))
        fvpool = ctx.enter_context(tc.tile_pool(name="fvpool", bufs=4))
        work = ctx.enter_context(tc.tile_pool(name="work", bufs=3))
        small = ctx.enter_context(tc.tile_pool(name="small", bufs=4))
        ps_tp = ctx.enter_context(tc.tile_pool(name="ps_tp", bufs=2, space="PSUM"))
        ps_k = ctx.enter_context(tc.tile_pool(name="ps_k", bufs=3, space="PSUM"))
        ps_e = ctx.enter_context(tc.tile_pool(name="ps_e", bufs=1, space="PSUM"))
        ps_mm = ctx.enter_context(tc.tile_pool(name="ps_mm", bufs=2, space="PSUM"))
        dram = ctx.enter_context(tc.tile_pool(name="dram", bufs=2, space="DRAM"))

        # ---- param DMAs first: they ride the hw queue and must land
        # before the fv firehose saturates the DMA engines ----
        wa_nats = []
        for at in range(2):
            t = ldpool.tile([128, R], f32, tag=f"ldw{at}", name="ld")
            nc.sync.dma_start(out=t, in_=Wa[at * 128:(at + 1) * 128, :])
            wa_nats.append(t)
        ua_nats = []
        for at in range(2):
            t = ldpool.tile([128, M], f32, tag=f"ldu{at}", name="ld")
            nc.sync.dma_start(out=t, in_=Ua[at * 128:(at + 1) * 128, :])
            ua_nats.append(t)
        # hs natural [8, 512]: one contiguous 16 KB DMA (the old per-column
        # 4-byte gather took tens of us when DMA engines were busy)
        hs_nat = singles.tile([BLOC, R], f32, tag="hs_nat", name="hs_nat")
        nc.sync.dma_start(out=hs_nat, in_=hs)
        ba_b = singles.tile([BLOC, A], f32, tag="ba", name="ba")
        nc.sync.dma_start(out=ba_b,
                          in_=bass.AP(tensor=ba.tensor, offset=0,
                                      ap=[[0, BLOC], [1, A]]))

        # batch 0's fv cast-load, split in four so the prologue transposes
        # chase the arriving 1 MB chunks instead of one 4 MB completion
        fv_tiles = {}

        def fv_load(b, t0, nt):
            if b not in fv_tiles:
                fv_tiles[b] = fvpool.tile([128, NL, M], f16, tag="fv",
                                          name="fv")
            src = bass.AP(tensor=fv.tensor, offset=b * L * M + t0 * 128 * M,
                          ap=[[M, 128], [128 * M, nt], [1, M]])
            nc.gpsimd.dma_start(out=fv_tiles[b][:, t0:t0 + nt, :], in_=src)

        # identities BEFORE the fv piece loads: they gate the Wa/Ua/fv
        # transposes and the gpsimd queue is serial, so emitting them after
        # the descgens left PE idle until t~14us
        ident = singles.tile([128, 128], f32, tag="ident", name="ident")
        make_identity(nc, ident)
        ident16 = singles.tile([128, 128], f16, tag="ident16", name="ident16")
        make_identity(nc, ident16)

        ones_row = singles.tile([1, 128], f32, tag="ones_row", name="ones_row")
        nc.vector.memset(ones_row, 1.0)
        ones_col = singles.tile([128, 1], f32, tag="ones_col", name="ones_col")
        nc.vector.memset(ones_col, 1.0)

        # graduated pieces: a small first piece starts PE ~2us earlier (the
        # ~10.5us to first SWDGE packet is mostly fixed boot+descgen cost,
        # but the transfer tail is proportional)
        for t0, nt in ((0, 2), (2, 6), (8, 8), (16, 8), (24, 8)):
            fv_load(0, t0, nt)

        # ---- parameters into contraction-major layouts ----
        # WaT[rt] [128(r), 256(a)] fp32: WaT[rt][k, a] = Wa[a, 128*rt + k]
        WaT = [singles.tile([128, A], f32, tag=f"WaT{rt}", name=f"WaT{rt}")
               for rt in range(4)]
        # param PSUM evacuations ride ScalarE (idle in the prologue); on DVE
        # they backlogged the queue and delayed the first fvT copies ~2us
        for at in range(2):
            for rt in range(4):
                ps = ps_mm.tile([128, 128], f32, tag="mm", name="mm")
                nc.tensor.transpose(ps, wa_nats[at][:, rt * 128:(rt + 1) * 128],
                                    ident)
                nc.scalar.activation(
                    out=WaT[rt][:, at * 128:(at + 1) * 128], in_=ps,
                    func=AF.Copy, scale=1.0)
        # UaT[mh] [128(m), 256(a)] fp16: UaT[mh][k, a] = Ua[a, 128*mh + k]
        UaT = [singles.tile([128, A], f16, tag=f"UaT{mh}", name=f"UaT{mh}")
               for mh in range(2)]
        for at in range(2):
            for mh in range(2):
                ps = ps_mm.tile([128, 128], f32, tag="mm", name="mm")
                nc.tensor.transpose(ps, ua_nats[at][:, mh * 128:(mh + 1) * 128],
                                    ident)
                nc.scalar.activation(
                    out=UaT[mh][:, at * 128:(at + 1) * 128], in_=ps,
                    func=AF.Copy, scale=1.0)
        # w as fp16 stationary columns [128, 1] per a-half (cast during DMA)
        w_sb = [singles.tile([128, 1], f16, tag=f"w{ah}", name=f"w{ah}")
                for ah in range(2)]
        for ah in range(2):
            nc.gpsimd.dma_start(out=w_sb[ah], in_=w[ah * 128:(ah + 1) * 128, :])

        # hsT[rt] [128(r), BLOC] fp32 via PE transposes of hs_nat
        hsT = [singles.tile([128, BLOC], f32, tag=f"hsT{rt}", name=f"hsT{rt}")
               for rt in range(4)]
        for rt in range(4):
            ps = ps_mm.tile([128, BLOC], f32, tag="mm", name="mm")
            nc.tensor.transpose(ps, hs_nat[:, rt * 128:(rt + 1) * 128],
                                ident[:BLOC, :BLOC])
            nc.scalar.activation(out=hsT[rt], in_=ps, func=AF.Copy, scale=1.0)

        # q = hs @ Wa.T + ba   -> [BLOC, A] fp32
        q_ps = ps_mm.tile([BLOC, A], f32, tag="mm", name="mm")
        for rt in range(4):
            nc.tensor.matmul(q_ps, lhsT=hsT[rt], rhs=WaT[rt],
                             start=(rt == 0), stop=(rt == 3))
        q_sb = singles.tile([BLOC, A], f32, tag="q", name="q")
        nc.vector.tensor_add(q_sb, q_ps, ba_b)
        # qT[ah] [128(a), BLOC] fp32
        qT = [singles.tile([128, BLOC], f32, tag=f"qT{ah}", name=f"qT{ah}")
              for ah in range(2)]
        for ah in range(2):
            ps = ps_mm.tile([128, BLOC], f32, tag="mm", name="mm")
            nc.tensor.transpose(ps, q_sb[:, ah * 128:(ah + 1) * 128],
                                ident[:BLOC, :BLOC])
            nc.vector.tensor_copy(out=qT[ah], in_=ps)

        # ---- main per-batch-row pipeline ----
        def main_stage(b):
            fv_nat = fv_tiles[b]
            # emit row b+1's load BEFORE this row's all-reduces join the
            # gpsimd queue: the load then starts a full row early instead
            # of after AR(b) completes mid-row b+1 (zero-margin chase that
            # caused multi-us PE starvation when anything hiccupped)
            if b + 1 < BLOC:
                fv_load(b + 1, 0, NL)

            e_sb = small.tile([1, L], f32, tag="e_sb", name="e_sb")
            e_d = dram.tile([L], f32, tag="e_d", name="e_d")

            # software-pipelined j-groups: PE emits transposes(i),
            # k-matmuls(i-1), e-matmul(i-2) per step so it never stalls on
            # the DVE fvT-copy or ACT tanh of the current group.
            fvT_q = {}
            t_q = {}

            def emit_T(jg):
                # both m-halves' transposes share one fp16 PSUM bank
                pst = ps_tp.tile([128, 2, JW], f16, tag="tp", name="tp")
                for mh in range(2):
                    for c in range(4):
                        t = jg * 4 + c
                        nc.tensor.transpose(
                            pst[:, mh, c * 128:(c + 1) * 128],
                            fv_nat[:, t, mh * 128:(mh + 1) * 128], ident16)
                # one [128, 1024] evacuation instead of two [128, 512]s:
                # fewer DVE issues and one semaphore for the k-matmuls
                fvT = work.tile([128, 2, JW], f16, tag="fvT", name="fvT")
                nc.vector.tensor_copy(out=fvT, in_=pst)
                fvT_q[jg] = fvT

            def emit_K(jg):
                fvT = fvT_q.pop(jg)
                t_sb = [work.tile([128, JW], f16, tag=f"t{ah}", name=f"t{ah}")
                        for ah in range(2)]
                for ah in range(2):
                    psk = ps_k.tile([128, JW], f32, tag="kk", name="kk")
                    for mh in range(2):
                        nc.tensor.matmul(
                            psk, lhsT=UaT[mh][:, ah * 128:(ah + 1) * 128],
                            rhs=fvT[:, mh, :], start=(mh == 0), stop=(mh == 1))
                    nc.scalar.activation(out=t_sb[ah], in_=psk, func=AF.Tanh,
                                         bias=qT[ah][:, b:b + 1], scale=1.0)
                t_q[jg] = t_sb

            # e [1, 4096] -> e_t [128, 32] with e_t[p, t] = e[128*t + p]:
            # direct SBUF->SBUF partition-scatter DMA, two halves per row
            # pipelined inside the row (no DRAM round-trip, 2 sync-queue
            # issues per row instead of 10)
            e_t = small.tile([128, NL], f32, tag="e_t", name="e_t")

            def bounce_piece(h, npieces):
                PW = L // npieces
                PT = NL // npieces
                nc.sync.dma_start(
                    out=bass.AP(tensor=e_d.tensor, offset=e_d.offset + h * PW,
                                ap=[[0, 1], [1, PW]]),
                    in_=e_sb[:, h * PW:(h + 1) * PW])
                nc.sync.dma_start(
                    out=e_t[:, h * PT:(h + 1) * PT],
                    in_=bass.AP(tensor=e_d.tensor, offset=e_d.offset + h * PW,
                                ap=[[1, 128], [128, PT]]))

            def emit_E(jg):
                t_sb = t_q.pop(jg)
                pse = ps_e.tile([1, JW], f32, tag="ee", name="ee")
                for ah in range(2):
                    nc.tensor.matmul(pse, lhsT=w_sb[ah], rhs=t_sb[ah],
                                     start=(ah == 0), stop=(ah == 1))
                nc.vector.tensor_copy(out=e_sb[:, jg * JW:(jg + 1) * JW],
                                      in_=pse)
                if b == BLOC - 1:
                    pass  # tail row: full PE reshape below, zero DMA waits
                elif b <= 1:
                    # early rows: DMA-bounce only the first half (quarters
                    # written right after jg 1 and 3); second half via PE
                    if jg in (1, 3):
                        bounce_piece(jg // 2, 4)
                elif jg == NJG // 2 - 1 or jg == NJG - 1:
                    bounce_piece(0 if jg == NJG // 2 - 1 else 1, 2)

            for i in range(NJG + 2):
                if i < NJG:
                    emit_T(i)
                if 1 <= i:
                    emit_K(i - 1) if (i - 1) < NJG else None
                if 2 <= i:
                    emit_E(i - 2)

            return fv_nat, e_sb, e_t

        # softmax_part(b) is emitted only AFTER main_stage(b+1): the DVE
        # reduce_max waits on the bounce readback, and emitting it inline
        # head-of-line-blocked the in-order DVE queue (row b+1's fvT copies
        # queued behind it, starving the PE k-matmuls for ~6us whenever the
        # readback ran late under fv-load DMA congestion)
        def softmax_part(b, e_sb, e_t):
            last = (b == BLOC - 1)
            if b <= 1 or last:
                # e tail -> e_t columns via PE column transposes (LDW + 1
                # col each, ~85 ns) straight from e_sb with zero DMA
                # latency: second half for rows 0-1, ALL of it for the tail
                # row (whose readbacks would otherwise gate the epilogue)
                lo = 0 if last else 16
                n = NL - lo
                ps_et = ps_mm.tile([128, n], f32, tag="mm", name="mm")
                for c in range(n):
                    nc.tensor.transpose(
                        ps_et[:, c:c + 1],
                        e_sb[:, (lo + c) * 128:(lo + c + 1) * 128],
                        ident[:1, :1])
                nc.vector.tensor_copy(out=e_t[:, lo:], in_=ps_et)
            mrow = small.tile([128, 1], f32, tag="mrow", name="mrow")
            nc.vector.reduce_max(out=mrow, in_=e_t, axis=mybir.AxisListType.X)
            negm = small.tile([128, 1], f32, tag="negm", name="negm")
            if not last:
                # steady state: gpsimd all-reduce runs concurrently with PE
                mall = small.tile([128, 1], f32, tag="mall", name="mall")
                nc.gpsimd.partition_all_reduce(mall, mrow, channels=128,
                                               reduce_op=bass_isa.ReduceOp.max)
                nc.vector.tensor_scalar_mul(negm, mall, -1.0)
            else:
                # tail row: PE is idle here, and the PE/DVE chain has much
                # lower latency than two gpsimd ucode launches
                ps_m = ps_mm.tile([1, 128], f32, tag="mm", name="mm")
                nc.tensor.transpose(ps_m, mrow, ident)
                mg = small.tile([1, 1], f32, tag="mg", name="mg")
                nc.vector.reduce_max(out=mg, in_=ps_m,
                                     axis=mybir.AxisListType.X)
                negmg = small.tile([1, 1], f32, tag="negmg", name="negmg")
                nc.vector.tensor_scalar_mul(negmg, mg, -1.0)
                ps_b = ps_mm.tile([128, 1], f32, tag="mm", name="mm")
                nc.tensor.matmul(ps_b, lhsT=ones_row, rhs=negmg,
                                 start=True, stop=True)
                nc.vector.tensor_copy(out=negm, in_=ps_b)
            p_t = small.tile([128, NL], f16, tag="p_t", name="p_t")
            srow = small.tile([128, 1], f32, tag="srow", name="srow")
            nc.scalar.activation(out=p_t, in_=e_t, func=AF.Exp, bias=negm,
                                 scale=1.0, accum_out=srow)
            rz = small.tile([1, 1], f32, tag="rz", name="rz")
            if not last:
                sall = small.tile([128, 1], f32, tag="sall", name="sall")
                nc.gpsimd.partition_all_reduce(sall, srow, channels=128,
                                               reduce_op=bass_isa.ReduceOp.add)
                nc.vector.reciprocal(out=rz, in_=sall[0:1, :])
            else:
                ps_s = ps_mm.tile([1, 1], f32, tag="mm", name="mm")
                nc.tensor.matmul(ps_s, lhsT=ones_col, rhs=srow,
                                 start=True, stop=True)
                nc.vector.reciprocal(out=rz, in_=ps_s)
            return p_t, rz

        # context rows accumulate in SBUF; ONE batched DMA at the end
        # (8 per-row 1KB writes cost ~0.8us of sync-queue time each, and
        # the last one's completion+semaphore gated the exit)
        ctx_all = singles.tile([1, BLOC * M], f32, tag="ctx_all",
                               name="ctx_all")

        # weighted sum, one batch row behind
        def ws_stage(b, fv_nat, p_t, rz):
            psw = ps_mm.tile([1, M], f32, tag="mm", name="mm")
            for t in range(NL):
                nc.tensor.matmul(psw, lhsT=p_t[:, t:t + 1], rhs=fv_nat[:, t, :],
                                 start=(t == 0), stop=(t == NL - 1))
            nc.vector.tensor_scalar_mul(ctx_all[:, b * M:(b + 1) * M],
                                        psw, rz)

        def finish_row(pb, pfv, pesb, pet):
            p_t, rz = softmax_part(pb, pesb, pet)
            ws_stage(pb, pfv, p_t, rz)

        # softmax+ws deferred 2 rows for every row: bounce readbacks can
        # run a full row late under fv-load DMA congestion, and with only
        # 1-row slack a late one head-of-line-blocks the DVE queue.  The
        # tail shape is unchanged (the final flush still ends f(6), f(7)).
        pending = []
        for b in range(BLOC):
            pending.append((b, main_stage(b)))
            while len(pending) > 2:
                pb, saved = pending.pop(0)
                finish_row(pb, *saved)
        for pb, saved in pending:
            finish_row(pb, *saved)
        nc.sync.dma_start(out=ctx_out, in_=ctx_all)

    nc.compile()
    return nc


def _get_nc():
    if "nc" not in _CACHE:
        _CACHE["nc"] = _build()
    return _CACHE["nc"]


def kernel(hidden_state, feature_vectors, Wa, Ua, w, ba):
    from concourse.bass_utils import run_bass_kernel_spmd

    nc = _get_nc()
    hidden_state = np.ascontiguousarray(hidden_state, dtype=np.float32)
    feature_vectors = np.ascontiguousarray(feature_vectors, dtype=np.float32)
    params = {
        "Wa": np.ascontiguousarray(Wa, dtype=np.float32),
        "Ua": np.ascontiguousarray(Ua, dtype=np.float32),
        "w": np.ascontiguousarray(w, dtype=np.float32),
        "ba": np.ascontiguousarray(ba, dtype=np.float32),
    }
    in_maps = [
        {
            "hidden_state": hidden_state[c * BLOC:(c + 1) * BLOC],
            "feature_vectors": feature_vectors[c * BLOC:(c + 1) * BLOC],
            **params,
        }
        for c in range(NCORES)
    ]
    res = run_bass_kernel_spmd(nc, in_maps, list(range(NCORES)))
    return np.concatenate([res.results[c]["context"] for c in range(NCORES)],
                          axis=0)

